# revision 34
# baseline (speedup 1.0000x reference)
"""Multi-head "channel attention" kernel for Trainium2 (8 NeuronCores).

Reference computation (B=16, D=512, N=2048, h=8 heads, Nh=256):
    q = Wq @ XQ ; k = Wk @ XK ; v = Wv @ XV          (per batch, (D,N))
    per head (N split into 8 chunks of 256):
      scores = q_h @ k_h^T / sqrt(Nh)                ((D,D), contract over Nh)
      p      = softmax(scores, axis=-1)
      o_h    = p @ v_h                               ((D,Nh), contract over D)
    attn = concat(o_h) ; out = Wo @ (XQ - attn)

Sharding: data-parallel over batch: 16 batches / 8 cores = 2 per core.
No collectives needed.

Per-core kernel strategy (v3):
  * Host passes W.T for all four weights so every matmul operand loads
    from DRAM in its natural layout; the OUTPUT is stored n-major
    ([B, N, D], i.e. out^T per batch) and transposed back on the host.
  * The value path (Wv, XV, p~) runs in fp8e4 with DoubleRow matmuls
    (2 contraction rows/cycle): its error only perturbs attn, which is
    ~7% of Z's magnitude (measured end-to-end rel err ~4.5e-3 vs the
    2e-2 gate).  p~ = exp(scores/16 - 3): the -3 bias keeps the fp8
    range safe (max ~118 < 448) and cancels in the softmax since the
    -1 columns accumulate the same factor into r.
  * Head-level software pipelining: iteration h runs
        QT(h), KT(h), V(h), scores(h)+exp(h), then O+outT block (h-1).
    The O block consumes pt/v/xq produced a FULL iteration earlier, so
    the PE never waits on the exps or the Z-write STTs (which lag their
    PSUM groups by ~600ns on Scalar/Vector).
  * O-matmul: lhsT = p~ tiles, rhs = V with two columns fixed at -1.0,
    so PSUM column 256 accumulates -r; reciprocal gives -1/r and one
    fused scalar_tensor_tensor does Z = XQ + O * (-1/r).
  * Output projection is n-major: outT[n, e] = sum_d Z[d, n] WoT[d, e],
    accumulated INCREMENTALLY in two held PSUM banks two steps behind
    the O-loop.  After the last O group only two copies + two 128-row
    (2KB-line) DMAs remain, so there is no end-of-kernel backlog.
  * Startup: priority-ordered chunked DMAs on the in-order SP queue
    (wq+xq first): per-queue FIFO makes issue order an effective
    bandwidth priority.  PE warmup matmuls on dummy data hold the HAM
    clock ramp through the initial DMA window.
  * f32 paths (QT/KT/scores/outT) use float32r: 1 cycle/row on the PE
    when the moving free dim >= 256.
"""

import sys

if "/opt/trn_rl_repo" not in sys.path:
    sys.path.insert(0, "/opt/trn_rl_repo")

import ml_dtypes
import numpy as np

import concourse.bass as bass
import concourse.tile as tile
from concourse import bacc, mybir
from concourse.bass_utils import run_bass_kernel_spmd

B_PER_CORE = 2
D = 512
N = 2048
H = 8
NH = N // H  # 256
PT = D // 128  # 4 partition tiles over D
HT = NH // 128  # 2 partition tiles over one head's n-range

F32 = mybir.dt.float32
F32R = mybir.dt.float32r
F8 = mybir.dt.float8e4
WARM_F = 130  # warmup matmul free size (even, small init cost)
VPAD = 16  # v tile col padding: DoubleRow k-tile stride must be %16 bytes

_NC_CACHE = None


def build_nc():
    nc = bacc.Bacc("TRN2", target_bir_lowering=False, debug=False)

    xq = nc.dram_tensor("xq", [B_PER_CORE, D, N], F32R, kind="ExternalInput").ap()
    xk = nc.dram_tensor("xk", [B_PER_CORE, D, N], F32R, kind="ExternalInput").ap()
    xv = nc.dram_tensor("xv", [B_PER_CORE, D, N], F8, kind="ExternalInput").ap()
    wqt = nc.dram_tensor("wqt", [D, D], F32R, kind="ExternalInput").ap()
    wkt = nc.dram_tensor("wkt", [D, D], F32R, kind="ExternalInput").ap()
    wvt = nc.dram_tensor("wvt", [D, D], F8, kind="ExternalInput").ap()
    wot = nc.dram_tensor("wot", [D, D], F32R, kind="ExternalInput").ap()
    # n-major output: out[b, n, e] = final[b, e, n]; host transposes back.
    out = nc.dram_tensor("out", [B_PER_CORE, N, D], F32, kind="ExternalOutput").ap()

    with tile.TileContext(nc) as tc:
        with (
            tc.tile_pool(name="wpool", bufs=1) as wpool,
            tc.tile_pool(name="zpool", bufs=2) as zpool,
            tc.tile_pool(name="xpool", bufs=3) as xpool,
            tc.tile_pool(name="qkpool", bufs=2) as qkpool,
            tc.tile_pool(name="vpool", bufs=2) as vpool,
            tc.tile_pool(name="ptpool", bufs=2) as ptpool,
            tc.tile_pool(name="opool", bufs=6) as opool,
            tc.tile_pool(name="rpool", bufs=6) as rpool,
            tc.tile_pool(name="psq", bufs=2, space="PSUM") as psq,
            tc.tile_pool(name="pss", bufs=2, space="PSUM") as pss,
            tc.tile_pool(name="pso", bufs=2, space="PSUM") as pso,
            tc.tile_pool(name="pst", bufs=1, space="PSUM") as pst,
        ):
            # Weights resident for the whole kernel: [p, it, o] = W.T[it*128+p, o]
            w_sb = {}
            w_dram = {"wq": wqt, "wk": wkt, "wv": wvt, "wo": wot}

            def load_w(name, parts=1):
                # parts>1 splits the load into multiple dma_starts: a single
                # dma_start's descriptors only spread over ~4-6 of the 16 DMA
                # queues, so chunking raises effective transfer bandwidth.
                w_sb[name] = wpool.tile(
                    [128, PT, D],
                    F8 if name == "wv" else F32R,
                    name=f"w_{name}",
                    tag=f"w_{name}",
                )
                src = w_dram[name].rearrange("(t p) o -> p t o", p=128)
                step = PT // parts
                for c in range(parts):
                    sl = slice(c * step, (c + 1) * step)
                    nc.sync.dma_start(out=w_sb[name][:, sl, :], in_=src[:, sl, :])

            x_b = {
                "xq": [xq[b].rearrange("(t p) n -> p t n", p=128) for b in range(B_PER_CORE)],
                "xk": [xk[b].rearrange("(t p) n -> p t n", p=128) for b in range(B_PER_CORE)],
                "xv": [xv[b].rearrange("(t p) n -> p t n", p=128) for b in range(B_PER_CORE)],
            }

            def load_x(b, h, nm, parts=1):
                ns_ = slice(h * NH, (h + 1) * NH)
                t = xpool.tile(
                    [128, PT, NH],
                    F8 if nm == "xv" else F32R,
                    name=f"{nm}_h",
                    tag=f"{nm}_h",
                )
                step = PT // parts
                for c in range(parts):
                    sl = slice(c * step, (c + 1) * step)
                    nc.sync.dma_start(
                        out=t[:, sl, :], in_=x_b[nm][b][:, sl, ns_]
                    )
                return t

            def load_head(b, h):
                return [load_x(b, h, nm) for nm in ("xq", "xk", "xv")]

            steps = [(b, h) for b in range(B_PER_CORE) for h in range(H)]
            head_tiles = {}

            warm = None

            def warmup(n):
                # Warmups allocate from pss (idle outside the scores phase)
                # so they never wait on the psq copy rotation.
                for _ in range(n):
                    ps_w = pss.tile([128, D], F32, name="ps_s", tag="ps_s")
                    nc.tensor.matmul(
                        ps_w[:, 0:WARM_F],
                        lhsT=warm[:, 0:128],
                        rhs=warm[:, 0:WARM_F],
                        start=True,
                        stop=True,
                    )

            def store_outT(b, h, jn, src_ps):
                """Copy one accumulated outT block to SBUF and DMA it out.

                The copy runs on Vector at the block end, where its recip/STT
                work is already done; Scalar owns the V casts + exps."""
                o_sb = opool.tile([128, D], F32, name="o_sb", tag="o_sb")
                nc.vector.tensor_copy(out=o_sb, in_=src_ps)
                n0 = h * NH + jn * 128
                nc.sync.dma_start(out=out[b, n0 : n0 + 128, :], in_=o_sb)

            def emit_o_block(ob, oh, o_pt, o_v, o_xq):
                """O-matmul + Z + incremental n-major outT for head (ob, oh).

                Called one iteration AFTER the head's pt/v/xq were produced,
                so none of its PE work waits on Scalar/Vector."""
                z_h = zpool.tile([128, PT, NH], F32R, name="z_h", tag="z_h")
                psT = [
                    pst.tile([128, D], F32, name=f"ps_t{jn}", tag=f"ps_t{jn}")
                    for jn in range(HT)
                ]

                def outT_mm(dt_, stop):
                    for jn in range(HT):
                        nc.tensor.matmul(
                            psT[jn],
                            lhsT=z_h[:, dt_, jn * 128 : (jn + 1) * 128],
                            rhs=w_sb["wo"][:, dt_, :],
                            start=(dt_ == 0),
                            stop=stop,
                        )

                for dt_ in range(PT):
                    ps_o = pso.tile([128, NH + 2], F32, name="ps_o", tag="ps_o")
                    for etp in range(PT // 2):
                        nc.tensor.matmul(
                            ps_o,
                            lhsT=o_pt[
                                :, 2 * etp : 2 * etp + 2, dt_ * 128 : (dt_ + 1) * 128
                            ],
                            rhs=o_v[:, 2 * etp : 2 * etp + 2, 0 : NH + 2],
                            start=(etp == 0),
                            stop=(etp == PT // 2 - 1),
                            perf_mode=mybir.MatmulPerfMode.DoubleRow,
                        )
                    recip = rpool.tile([128, 1], F32, name="recip", tag="recip")
                    nc.vector.reciprocal(recip, ps_o[:, NH : NH + 1])
                    nc.vector.scalar_tensor_tensor(
                        out=z_h[:, dt_, :],
                        in0=ps_o[:, 0:NH],
                        scalar=recip,
                        in1=o_xq[:, dt_, :].bitcast(F32),
                        op0=mybir.AluOpType.mult,
                        op1=mybir.AluOpType.add,
                    )
                    # outT runs TWO dt behind: the Z-write STT on Vector lags
                    # the O PSUM group by ~600ns.
                    if dt_ >= 2:
                        outT_mm(dt_ - 2, stop=False)
                outT_mm(PT - 2, stop=False)
                outT_mm(PT - 1, stop=True)
                for jn in range(HT):
                    store_outT(ob, oh, jn, psT[jn])

            prev_ctx = None

            for idx, (b, h) in enumerate(steps):
                ns = slice(h * NH, (h + 1) * NH)

                if idx == 0:
                    # PE warmup: matmuls on dummy data during the initial DMA
                    # window hold the HAM clock ramp (otherwise the first
                    # ~3.4us run at 1.2GHz and the ramp restarts on idle).
                    # The init activation is kept tiny (130 cols) so the first
                    # warmup matmul starts ~300ns after Scalar boots.
                    warm = wpool.tile([128, 130], F32R, name="warm", tag="warm")
                    nc.scalar.activation(
                        out=warm,
                        in_=warm.bitcast(F32),
                        func=mybir.ActivationFunctionType.Copy,
                        bias=0.0,
                        scale=0.0,
                    )
                    # [128,1] constant -3.0 used as the exp bias AP.
                    # exp(s/16 - 3) keeps the fp8e4 outputs below 448 (the
                    # observed max at -3 is ~118; at -1 the data hits ~870
                    # and a single overflow NaNs a whole head via r).
                    neg_b = wpool.tile([128, 1], F32, name="negb", tag="negb")
                    nc.scalar.activation(
                        out=neg_b,
                        in_=warm[:, 0:1].bitcast(F32),
                        func=mybir.ActivationFunctionType.Copy,
                        bias=-3.0,
                        scale=0.0,
                    )
                    warmup(20)
                    # Priority-ordered chunked startup DMAs: the SP queue is
                    # in-order and the DMA queues are FIFO, so issue order is
                    # an effective bandwidth priority.
                    w_sb["wq"] = wpool.tile(
                        [128, PT, D], F32R, name="w_wq", tag="w_wq"
                    )
                    wq_src = w_dram["wq"].rearrange("(t p) o -> p t o", p=128)
                    t_xq = xpool.tile([128, PT, NH], F32R, name="xq_h", tag="xq_h")
                    for c in range(2):
                        sl = slice(c * 2, (c + 1) * 2)
                        nc.sync.dma_start(
                            out=w_sb["wq"][:, sl, :], in_=wq_src[:, sl, :]
                        )
                        nc.sync.dma_start(
                            out=t_xq[:, sl, :], in_=x_b["xq"][0][:, sl, ns]
                        )
                    load_w("wk", parts=2)
                    t_xk = load_x(0, 0, "xk", parts=2)
                    load_w("wv", parts=2)
                    t_xv = load_x(0, 0, "xv", parts=2)
                    head_tiles[(0, 1)] = load_head(0, 1)
                    load_w("wo", parts=2)
                    head_tiles[(0, 0)] = [t_xq, t_xk, t_xv]

                xq_h, xk_h, xv_h = head_tiles.pop((b, h))
                # Prefetch the next head's inputs (heads 0 and 1 were
                # prefetched in the startup sequence).
                if idx >= 1 and idx + 1 < len(steps):
                    head_tiles[steps[idx + 1]] = load_head(*steps[idx + 1])

                # QT/KT: [p, jt, d] = X^T @ W^T  (n-major projections)
                qt_h = qkpool.tile([128, HT, D], F32R, name="qt_h", tag="qt_h")
                kt_h = qkpool.tile([128, HT, D], F32R, name="kt_h", tag="kt_h")
                for dst, src, w in ((qt_h, xq_h, "wq"), (kt_h, xk_h, "wk")):
                    for jt in range(HT):
                        ps = psq.tile([128, D], F32, name="ps_p", tag="ps_p")
                        for it in range(PT):
                            nc.tensor.matmul(
                                ps,
                                lhsT=src[:, it, jt * 128 : (jt + 1) * 128],
                                rhs=w_sb[w][:, it, :],
                                start=(it == 0),
                                stop=(it == PT - 1),
                            )
                        # Vector, not Scalar: Scalar must stay free for exps.
                        nc.vector.tensor_copy(out=dst[:, jt, :], in_=ps)
                    if idx == 0 and dst is qt_h:
                        # wk/xk still in flight during head 0's QT.
                        warmup(2)

                # V (d-major): [p, et, n] in fp8e4; columns NH/NH+1 fixed at
                # -1.0 so the O-matmul accumulates -r in PSUM column NH.
                # fp8 DoubleRow contracts 2 k-tiles (256 rows) per matmul.
                v_h = vpool.tile([128, PT, NH + VPAD], F8, name="v_h", tag="v_h")
                # memset can't emit fp8; ACT Copy(in*0 - 1) = -1.0 can.
                nc.scalar.activation(
                    out=v_h[:, :, NH : NH + 2],
                    in_=w_sb["wq"][:, :, 0:2].bitcast(F32),
                    func=mybir.ActivationFunctionType.Copy,
                    bias=-1.0,
                    scale=0.0,
                )

                def v_group(et):
                    ps = psq.tile([128, D], F32, name="ps_p", tag="ps_p")
                    for itp in range(PT // 2):
                        nc.tensor.matmul(
                            ps[:, 0:NH],
                            lhsT=w_sb["wv"][
                                :, 2 * itp : 2 * itp + 2, et * 128 : (et + 1) * 128
                            ],
                            rhs=xv_h[:, 2 * itp : 2 * itp + 2, :],
                            start=(itp == 0),
                            stop=(itp == PT // 2 - 1),
                            perf_mode=mybir.MatmulPerfMode.DoubleRow,
                        )
                    # Scalar, not Vector: the serialized Vector queue (qt/kt
                    # copies + recip/STT) otherwise delays these casts past
                    # the next head's psq-slot reuse.
                    nc.scalar.copy(out=v_h[:, et, 0:NH], in_=ps[:, 0:NH])

                # Head 0's V data lands after scores' inputs, so V runs after
                # scores there; otherwise V sits between KT and scores.
                if idx >= 1:
                    for et in range(PT):
                        v_group(et)

                # scoresT (e-part, d-free) then p~ = exp(scoresT/16 - 3) fp8
                pt_t = ptpool.tile([128, PT, D], F8, name="pt_t", tag="pt_t")
                for et in range(PT):
                    ps_s = pss.tile([128, D], F32, name="ps_s", tag="ps_s")
                    for jt in range(HT):
                        nc.tensor.matmul(
                            ps_s,
                            lhsT=kt_h[:, jt, et * 128 : (et + 1) * 128],
                            rhs=qt_h[:, jt, :],
                            start=(jt == 0),
                            stop=(jt == HT - 1),
                        )
                    nc.scalar.activation(
                        out=pt_t[:, et, :],
                        in_=ps_s,
                        func=mybir.ActivationFunctionType.Exp,
                        scale=float(1.0 / np.sqrt(NH)),
                        bias=neg_b,
                    )

                if idx == 0:
                    warmup(2)
                    for et in range(PT):
                        v_group(et)

                # O + Z + outT for the PREVIOUS head: its pt/v/xq finished a
                # full iteration ago, so the PE never stalls on them.
                if prev_ctx is not None:
                    emit_o_block(*prev_ctx)
                prev_ctx = (b, h, pt_t, v_h, xq_h)

            emit_o_block(*prev_ctx)

    nc.compile()
    return nc


def _get_nc():
    global _NC_CACHE
    if _NC_CACHE is None:
        _NC_CACHE = build_nc()
    return _NC_CACHE


def _shard_inputs(inputs):
    f8 = ml_dtypes.float8_e4m3
    xq = np.ascontiguousarray(np.asarray(inputs["X_Query"], dtype=np.float32))
    xk = np.ascontiguousarray(np.asarray(inputs["X_Key"], dtype=np.float32))
    xv = np.ascontiguousarray(
        np.asarray(inputs["X_Value"], dtype=np.float32).astype(f8)
    )
    weights = {
        "wqt": np.ascontiguousarray(np.asarray(inputs["W_q"], dtype=np.float32).T),
        "wkt": np.ascontiguousarray(np.asarray(inputs["W_k"], dtype=np.float32).T),
        "wvt": np.ascontiguousarray(
            np.asarray(inputs["W_v"], dtype=np.float32).T.astype(f8)
        ),
        "wot": np.ascontiguousarray(np.asarray(inputs["W_o"], dtype=np.float32).T),
    }
    in_maps = []
    for c in range(8):
        sl = slice(c * B_PER_CORE, (c + 1) * B_PER_CORE)
        in_maps.append(
            {"xq": xq[sl], "xk": xk[sl], "xv": xv[sl], **weights}
        )
    return in_maps


def run_sharded(inputs, **kwargs):
    """Run on all 8 cores; returns (full_output, BassKernelResults)."""
    nc = _get_nc()
    in_maps = _shard_inputs(inputs)
    res = run_bass_kernel_spmd(nc, in_maps, core_ids=list(range(8)), **kwargs)
    # per-core out is [B_PER_CORE, N, D] (n-major); transpose back.
    full = np.concatenate(
        [np.ascontiguousarray(r["out"].transpose(0, 2, 1)) for r in res.results],
        axis=0,
    )
    return full, res


def kernel(**inputs):
    full, _ = run_sharded(inputs)
    return full


# revision 39
# speedup vs baseline: 1.4765x; 1.4765x over previous
"""Multi-head "channel attention" kernel for Trainium2 (8 NeuronCores).

Reference computation (B=16, D=512, N=2048, h=8 heads, Nh=256):
    q = Wq @ XQ ; k = Wk @ XK ; v = Wv @ XV          (per batch, (D,N))
    per head (N split into 8 chunks of 256):
      scores = q_h @ k_h^T / sqrt(Nh)                ((D,D), contract over Nh)
      p      = softmax(scores, axis=-1)
      o_h    = p @ v_h                               ((D,Nh), contract over D)
    attn = concat(o_h) ; out = Wo @ (XQ - attn)

Sharding: data-parallel over batch: 16 batches / 8 cores = 2 per core.
No collectives needed.

Per-core kernel strategy (v3):
  * Host passes W.T for all four weights so every matmul operand loads
    from DRAM in its natural layout; the OUTPUT is stored n-major
    ([B, N, D], i.e. out^T per batch) and transposed back on the host.
  * The value path (Wv, XV, p~) runs in fp8e4 with DoubleRow matmuls
    (2 contraction rows/cycle): its error only perturbs attn, which is
    ~7% of Z's magnitude (measured end-to-end rel err ~4.5e-3 vs the
    2e-2 gate).  p~ = exp(scores/16 - 3): the -3 bias keeps the fp8
    range safe (max ~118 < 448) and cancels in the softmax since the
    -1 columns accumulate the same factor into r.
  * Head-level software pipelining: iteration h runs
        QT(h), KT(h), V(h), scores(h)+exp(h), then O+outT block (h-1).
    The O block consumes pt/v/xq produced a FULL iteration earlier, so
    the PE never waits on the exps or the Z-write STTs (which lag their
    PSUM groups by ~600ns on Scalar/Vector).
  * O-matmul: lhsT = p~ tiles, rhs = V with two columns fixed at -1.0,
    so PSUM column 256 accumulates -r; reciprocal gives -1/r and one
    fused scalar_tensor_tensor does Z = XQ + O * (-1/r).
  * Output projection is n-major: outT[n, e] = sum_d Z[d, n] WoT[d, e],
    accumulated INCREMENTALLY in two held PSUM banks two steps behind
    the O-loop.  After the last O group only two copies + two 128-row
    (2KB-line) DMAs remain, so there is no end-of-kernel backlog.
  * Startup: priority-ordered chunked DMAs on the in-order SP queue
    (wq+xq first): per-queue FIFO makes issue order an effective
    bandwidth priority.  PE warmup matmuls on dummy data hold the HAM
    clock ramp through the initial DMA window.
  * f32 paths (QT/KT/scores/outT) use float32r: 1 cycle/row on the PE
    when the moving free dim >= 256.
"""

import sys

if "/opt/trn_rl_repo" not in sys.path:
    sys.path.insert(0, "/opt/trn_rl_repo")

import ml_dtypes
import numpy as np

import concourse.bass as bass
import concourse.tile as tile
from concourse import bacc, mybir
from concourse.bass_utils import run_bass_kernel_spmd

B_PER_CORE = 2
D = 512
N = 2048
H = 8
NH = N // H  # 256
PT = D // 128  # 4 partition tiles over D
HT = NH // 128  # 2 partition tiles over one head's n-range

F32 = mybir.dt.float32
F32R = mybir.dt.float32r
F8 = mybir.dt.float8e4
WARM_F = 130  # warmup matmul free size (even, small init cost)
VPAD = 16  # v tile col padding: DoubleRow k-tile stride must be %16 bytes

_NC_CACHE = None


def build_nc():
    nc = bacc.Bacc("TRN2", target_bir_lowering=False, debug=False)

    # xq is loaded twice: f32r for the residual Z = XQ - attn, fp8 for the
    # QT projection (all of q/k/v/p~ run as fp8 DoubleRow matmuls; measured
    # end-to-end rel err ~7.6e-3 vs the 2e-2 gate).
    xq = nc.dram_tensor("xq", [B_PER_CORE, D, N], F32R, kind="ExternalInput").ap()
    xq8 = nc.dram_tensor("xq8", [B_PER_CORE, D, N], F8, kind="ExternalInput").ap()
    xk = nc.dram_tensor("xk", [B_PER_CORE, D, N], F8, kind="ExternalInput").ap()
    xv = nc.dram_tensor("xv", [B_PER_CORE, D, N], F8, kind="ExternalInput").ap()
    wqt = nc.dram_tensor("wqt", [D, D], F8, kind="ExternalInput").ap()
    wkt = nc.dram_tensor("wkt", [D, D], F8, kind="ExternalInput").ap()
    wvt = nc.dram_tensor("wvt", [D, D], F8, kind="ExternalInput").ap()
    wot = nc.dram_tensor("wot", [D, D], F32R, kind="ExternalInput").ap()
    # n-major output: out[b, n, e] = final[b, e, n]; host transposes back.
    out = nc.dram_tensor("out", [B_PER_CORE, N, D], F32, kind="ExternalOutput").ap()

    with tile.TileContext(nc) as tc:
        with (
            tc.tile_pool(name="wpool", bufs=1) as wpool,
            tc.tile_pool(name="zpool", bufs=2) as zpool,
            tc.tile_pool(name="xpool", bufs=3) as xpool,
            tc.tile_pool(name="qkpool", bufs=2) as qkpool,
            tc.tile_pool(name="vpool", bufs=2) as vpool,
            tc.tile_pool(name="ptpool", bufs=2) as ptpool,
            tc.tile_pool(name="opool", bufs=6) as opool,
            tc.tile_pool(name="rpool", bufs=6) as rpool,
            tc.tile_pool(name="psq", bufs=2, space="PSUM") as psq,
            tc.tile_pool(name="pss", bufs=2, space="PSUM") as pss,
            tc.tile_pool(name="pso", bufs=2, space="PSUM") as pso,
            tc.tile_pool(name="pst", bufs=1, space="PSUM") as pst,
        ):
            # Weights resident for the whole kernel: [p, it, o] = W.T[it*128+p, o]
            w_sb = {}
            w_dram = {"wq": wqt, "wk": wkt, "wv": wvt, "wo": wot}

            def load_w(name, parts=1):
                # parts>1 splits the load into multiple dma_starts: a single
                # dma_start's descriptors only spread over ~4-6 of the 16 DMA
                # queues, so chunking raises effective transfer bandwidth.
                w_sb[name] = wpool.tile(
                    [128, PT, D],
                    F32R if name == "wo" else F8,
                    name=f"w_{name}",
                    tag=f"w_{name}",
                )
                src = w_dram[name].rearrange("(t p) o -> p t o", p=128)
                step = PT // parts
                for c in range(parts):
                    sl = slice(c * step, (c + 1) * step)
                    nc.sync.dma_start(out=w_sb[name][:, sl, :], in_=src[:, sl, :])

            x_b = {
                "xq": [xq[b].rearrange("(t p) n -> p t n", p=128) for b in range(B_PER_CORE)],
                "xq8": [xq8[b].rearrange("(t p) n -> p t n", p=128) for b in range(B_PER_CORE)],
                "xk": [xk[b].rearrange("(t p) n -> p t n", p=128) for b in range(B_PER_CORE)],
                "xv": [xv[b].rearrange("(t p) n -> p t n", p=128) for b in range(B_PER_CORE)],
            }

            def load_x(b, h, nm, parts=1):
                ns_ = slice(h * NH, (h + 1) * NH)
                t = xpool.tile(
                    [128, PT, NH],
                    F32R if nm == "xq" else F8,
                    name=f"{nm}_h",
                    tag=f"{nm}_h",
                )
                step = PT // parts
                for c in range(parts):
                    sl = slice(c * step, (c + 1) * step)
                    nc.sync.dma_start(
                        out=t[:, sl, :], in_=x_b[nm][b][:, sl, ns_]
                    )
                return t

            def load_head(b, h):
                return [load_x(b, h, nm) for nm in ("xq", "xq8", "xk", "xv")]

            steps = [(b, h) for b in range(B_PER_CORE) for h in range(H)]
            head_tiles = {}

            warm = None

            def warmup(n):
                # Warmups allocate from pss (idle outside the scores phase)
                # so they never wait on the psq copy rotation.
                for _ in range(n):
                    ps_w = pss.tile([128, D], F32, name="ps_s", tag="ps_s")
                    nc.tensor.matmul(
                        ps_w[:, 0:WARM_F],
                        lhsT=warm[:, 0:128],
                        rhs=warm[:, 0:WARM_F],
                        start=True,
                        stop=True,
                    )

            def store_outT(b, h, jn, src_ps):
                """Copy one accumulated outT block to SBUF and DMA it out.

                The copy runs on Scalar: Vector is saturated with recip/STT
                during the O-loop, Scalar is idle there."""
                o_sb = opool.tile([128, D], F32, name="o_sb", tag="o_sb")
                nc.scalar.copy(out=o_sb, in_=src_ps)
                n0 = h * NH + jn * 128
                nc.sync.dma_start(out=out[b, n0 : n0 + 128, :], in_=o_sb)

            def emit_o_block(ob, oh, o_pt, o_v, o_xq):
                """O-matmul + Z + incremental n-major outT for head (ob, oh).

                Called one iteration AFTER the head's pt/v/xq were produced,
                so none of its PE work waits on Scalar/Vector."""
                z_h = zpool.tile([128, PT, NH], F32R, name="z_h", tag="z_h")
                psT = [
                    pst.tile([128, D], F32, name=f"ps_t{jn}", tag=f"ps_t{jn}")
                    for jn in range(HT)
                ]

                def outT_mm(dt_, stop):
                    for jn in range(HT):
                        nc.tensor.matmul(
                            psT[jn],
                            lhsT=z_h[:, dt_, jn * 128 : (jn + 1) * 128],
                            rhs=w_sb["wo"][:, dt_, :],
                            start=(dt_ == 0),
                            stop=stop,
                        )

                for dt_ in range(PT):
                    ps_o = pso.tile([128, NH + 2], F32, name="ps_o", tag="ps_o")
                    for etp in range(PT // 2):
                        nc.tensor.matmul(
                            ps_o,
                            lhsT=o_pt[
                                :, 2 * etp : 2 * etp + 2, dt_ * 128 : (dt_ + 1) * 128
                            ],
                            rhs=o_v[:, 2 * etp : 2 * etp + 2, 0 : NH + 2],
                            start=(etp == 0),
                            stop=(etp == PT // 2 - 1),
                            perf_mode=mybir.MatmulPerfMode.DoubleRow,
                        )
                    recip = rpool.tile([128, 1], F32, name="recip", tag="recip")
                    nc.vector.reciprocal(recip, ps_o[:, NH : NH + 1])
                    nc.vector.scalar_tensor_tensor(
                        out=z_h[:, dt_, :],
                        in0=ps_o[:, 0:NH],
                        scalar=recip,
                        in1=o_xq[:, dt_, :].bitcast(F32),
                        op0=mybir.AluOpType.mult,
                        op1=mybir.AluOpType.add,
                    )
                    # outT runs TWO dt behind: the Z-write STT on Vector lags
                    # the O PSUM group by ~600ns.
                    if dt_ >= 2:
                        outT_mm(dt_ - 2, stop=False)
                outT_mm(PT - 2, stop=False)
                outT_mm(PT - 1, stop=True)
                for jn in range(HT):
                    store_outT(ob, oh, jn, psT[jn])

            prev_ctx = None

            for idx, (b, h) in enumerate(steps):
                ns = slice(h * NH, (h + 1) * NH)

                if idx == 0:
                    # PE warmup: matmuls on dummy data during the initial DMA
                    # window hold the HAM clock ramp (otherwise the first
                    # ~3.4us run at 1.2GHz and the ramp restarts on idle).
                    # The init activation is kept tiny (130 cols) so the first
                    # warmup matmul starts ~300ns after Scalar boots.
                    warm = wpool.tile([128, 130], F32R, name="warm", tag="warm")
                    nc.scalar.activation(
                        out=warm,
                        in_=warm.bitcast(F32),
                        func=mybir.ActivationFunctionType.Copy,
                        bias=0.0,
                        scale=0.0,
                    )
                    # [128,1] constant -3.0 used as the exp bias AP.
                    # exp(s/16 - 3) keeps the fp8e4 outputs below 448 (the
                    # observed max at -3 is ~118; at -1 the data hits ~870
                    # and a single overflow NaNs a whole head via r).
                    neg_b = wpool.tile([128, 1], F32, name="negb", tag="negb")
                    nc.scalar.activation(
                        out=neg_b,
                        in_=warm[:, 0:1].bitcast(F32),
                        func=mybir.ActivationFunctionType.Copy,
                        bias=-3.0,
                        scale=0.0,
                    )
                    warmup(7)
                    # Priority-ordered chunked startup DMAs: the SP queue is
                    # in-order and the DMA queues are FIFO, so issue order is
                    # an effective bandwidth priority.  The fp8 weights are
                    # only 256KB, so QT's inputs (wq8+xq8 = 384KB) land
                    # ~1.1us after the first line.  xq (f32r, residual) is
                    # only needed by the O-block one iteration later.
                    load_w("wq", parts=2)
                    t_xq8 = load_x(0, 0, "xq8", parts=2)
                    load_w("wk", parts=2)
                    t_xk = load_x(0, 0, "xk", parts=2)
                    load_w("wv", parts=2)
                    t_xv = load_x(0, 0, "xv", parts=2)
                    t_xq = load_x(0, 0, "xq", parts=2)
                    head_tiles[(0, 1)] = load_head(0, 1)
                    load_w("wo", parts=2)
                    head_tiles[(0, 0)] = [t_xq, t_xq8, t_xk, t_xv]

                xq_h, xq8_h, xk_h, xv_h = head_tiles.pop((b, h))
                # Prefetch the next head's inputs (heads 0 and 1 were
                # prefetched in the startup sequence).
                if idx >= 1 and idx + 1 < len(steps):
                    head_tiles[steps[idx + 1]] = load_head(*steps[idx + 1])

                # QT/KT: [p, jt, d] = X^T @ W^T (n-major projections), fp8
                # DoubleRow: each matmul contracts an it-pair (256 rows).
                qt_h = qkpool.tile([128, HT, D], F8, name="qt_h", tag="qt_h")
                kt_h = qkpool.tile([128, HT, D], F8, name="kt_h", tag="kt_h")
                for dst, xsrc, w in ((qt_h, xq8_h, "wq"), (kt_h, xk_h, "wk")):
                    for jt in range(HT):
                        ps = psq.tile([128, D], F32, name="ps_p", tag="ps_p")
                        for itp in range(PT // 2):
                            nc.tensor.matmul(
                                ps,
                                lhsT=xsrc[
                                    :, 2 * itp : 2 * itp + 2, jt * 128 : (jt + 1) * 128
                                ],
                                rhs=w_sb[w][:, 2 * itp : 2 * itp + 2, :],
                                start=(itp == 0),
                                stop=(itp == PT // 2 - 1),
                                perf_mode=mybir.MatmulPerfMode.DoubleRow,
                            )
                        # Vector, not Scalar: Scalar must stay free for exps.
                        nc.vector.tensor_copy(out=dst[:, jt, :], in_=ps)
                    if idx == 0 and dst is qt_h:
                        # wk/xk still in flight during head 0's QT.
                        warmup(2)

                # V (d-major): [p, et, n] in fp8e4; columns NH/NH+1 fixed at
                # -1.0 so the O-matmul accumulates -r in PSUM column NH.
                # fp8 DoubleRow contracts 2 k-tiles (256 rows) per matmul.
                v_h = vpool.tile([128, PT, NH + VPAD], F8, name="v_h", tag="v_h")
                # memset can't emit fp8; ACT Copy(in*0 - 1) = -1.0 can.
                nc.scalar.activation(
                    out=v_h[:, :, NH : NH + 2],
                    in_=warm[:, 0:8].rearrange("p (a b) -> p a b", a=PT).bitcast(F32),
                    func=mybir.ActivationFunctionType.Copy,
                    bias=-1.0,
                    scale=0.0,
                )

                def v_group(et):
                    ps = psq.tile([128, D], F32, name="ps_p", tag="ps_p")
                    for itp in range(PT // 2):
                        nc.tensor.matmul(
                            ps[:, 0:NH],
                            lhsT=w_sb["wv"][
                                :, 2 * itp : 2 * itp + 2, et * 128 : (et + 1) * 128
                            ],
                            rhs=xv_h[:, 2 * itp : 2 * itp + 2, :],
                            start=(itp == 0),
                            stop=(itp == PT // 2 - 1),
                            perf_mode=mybir.MatmulPerfMode.DoubleRow,
                        )
                    nc.vector.tensor_copy(out=v_h[:, et, 0:NH], in_=ps[:, 0:NH])

                # Head 0's V data lands after scores' inputs, so V runs after
                # scores there; otherwise V sits between KT and scores.
                if idx >= 1:
                    for et in range(PT):
                        v_group(et)

                # scoresT (e-part, d-free) then p~ = exp(scoresT/16 - 3) fp8
                pt_t = ptpool.tile([128, PT, D], F8, name="pt_t", tag="pt_t")
                for et in range(PT):
                    ps_s = pss.tile([128, D], F32, name="ps_s", tag="ps_s")
                    nc.tensor.matmul(
                        ps_s,
                        lhsT=kt_h[:, 0:HT, et * 128 : (et + 1) * 128],
                        rhs=qt_h[:, 0:HT, :],
                        start=True,
                        stop=True,
                        perf_mode=mybir.MatmulPerfMode.DoubleRow,
                    )
                    nc.scalar.activation(
                        out=pt_t[:, et, :],
                        in_=ps_s,
                        func=mybir.ActivationFunctionType.Exp,
                        scale=float(1.0 / np.sqrt(NH)),
                        bias=neg_b,
                    )

                if idx == 0:
                    warmup(2)
                    for et in range(PT):
                        v_group(et)

                # O + Z + outT for the PREVIOUS head: its pt/v/xq finished a
                # full iteration ago, so the PE never stalls on them.
                if prev_ctx is not None:
                    emit_o_block(*prev_ctx)
                prev_ctx = (b, h, pt_t, v_h, xq_h)

            emit_o_block(*prev_ctx)

    nc.compile()
    return nc


def _get_nc():
    global _NC_CACHE
    if _NC_CACHE is None:
        _NC_CACHE = build_nc()
    return _NC_CACHE


def _shard_inputs(inputs):
    f8 = ml_dtypes.float8_e4m3
    xq = np.ascontiguousarray(np.asarray(inputs["X_Query"], dtype=np.float32))
    xq8 = np.ascontiguousarray(xq.astype(f8))
    xk = np.ascontiguousarray(
        np.asarray(inputs["X_Key"], dtype=np.float32).astype(f8)
    )
    xv = np.ascontiguousarray(
        np.asarray(inputs["X_Value"], dtype=np.float32).astype(f8)
    )
    weights = {
        "wqt": np.ascontiguousarray(
            np.asarray(inputs["W_q"], dtype=np.float32).T.astype(f8)
        ),
        "wkt": np.ascontiguousarray(
            np.asarray(inputs["W_k"], dtype=np.float32).T.astype(f8)
        ),
        "wvt": np.ascontiguousarray(
            np.asarray(inputs["W_v"], dtype=np.float32).T.astype(f8)
        ),
        "wot": np.ascontiguousarray(np.asarray(inputs["W_o"], dtype=np.float32).T),
    }
    in_maps = []
    for c in range(8):
        sl = slice(c * B_PER_CORE, (c + 1) * B_PER_CORE)
        in_maps.append(
            {"xq": xq[sl], "xq8": xq8[sl], "xk": xk[sl], "xv": xv[sl], **weights}
        )
    return in_maps


def run_sharded(inputs, **kwargs):
    """Run on all 8 cores; returns (full_output, BassKernelResults)."""
    nc = _get_nc()
    in_maps = _shard_inputs(inputs)
    res = run_bass_kernel_spmd(nc, in_maps, core_ids=list(range(8)), **kwargs)
    # per-core out is [B_PER_CORE, N, D] (n-major); transpose back.
    full = np.concatenate(
        [np.ascontiguousarray(r["out"].transpose(0, 2, 1)) for r in res.results],
        axis=0,
    )
    return full, res


def kernel(**inputs):
    full, _ = run_sharded(inputs)
    return full


# revision 40
# speedup vs baseline: 1.5214x; 1.0304x over previous
"""Multi-head "channel attention" kernel for Trainium2 (8 NeuronCores).

Reference computation (B=16, D=512, N=2048, h=8 heads, Nh=256):
    q = Wq @ XQ ; k = Wk @ XK ; v = Wv @ XV          (per batch, (D,N))
    per head (N split into 8 chunks of 256):
      scores = q_h @ k_h^T / sqrt(Nh)                ((D,D), contract over Nh)
      p      = softmax(scores, axis=-1)
      o_h    = p @ v_h                               ((D,Nh), contract over D)
    attn = concat(o_h) ; out = Wo @ (XQ - attn)

Sharding: data-parallel over batch: 16 batches / 8 cores = 2 per core.
No collectives needed.

Per-core kernel strategy (v3):
  * Host passes W.T for all four weights so every matmul operand loads
    from DRAM in its natural layout; the OUTPUT is stored n-major
    ([B, N, D], i.e. out^T per batch) and transposed back on the host.
  * The value path (Wv, XV, p~) runs in fp8e4 with DoubleRow matmuls
    (2 contraction rows/cycle): its error only perturbs attn, which is
    ~7% of Z's magnitude (measured end-to-end rel err ~4.5e-3 vs the
    2e-2 gate).  p~ = exp(scores/16 - 3): the -3 bias keeps the fp8
    range safe (max ~118 < 448) and cancels in the softmax since the
    -1 columns accumulate the same factor into r.
  * Head-level software pipelining: iteration h runs
        QT(h), KT(h), V(h), scores(h)+exp(h), then O+outT block (h-1).
    The O block consumes pt/v/xq produced a FULL iteration earlier, so
    the PE never waits on the exps or the Z-write STTs (which lag their
    PSUM groups by ~600ns on Scalar/Vector).
  * O-matmul: lhsT = p~ tiles, rhs = V with two columns fixed at -1.0,
    so PSUM column 256 accumulates -r; reciprocal gives -1/r and one
    fused scalar_tensor_tensor does Z = XQ + O * (-1/r).
  * Output projection is n-major: outT[n, e] = sum_d Z[d, n] WoT[d, e],
    accumulated INCREMENTALLY in two held PSUM banks two steps behind
    the O-loop.  After the last O group only two copies + two 128-row
    (2KB-line) DMAs remain, so there is no end-of-kernel backlog.
  * Startup: priority-ordered chunked DMAs on the in-order SP queue
    (wq+xq first): per-queue FIFO makes issue order an effective
    bandwidth priority.  PE warmup matmuls on dummy data hold the HAM
    clock ramp through the initial DMA window.
  * f32 paths (QT/KT/scores/outT) use float32r: 1 cycle/row on the PE
    when the moving free dim >= 256.
"""

import sys

if "/opt/trn_rl_repo" not in sys.path:
    sys.path.insert(0, "/opt/trn_rl_repo")

import ml_dtypes
import numpy as np

import concourse.bass as bass
import concourse.tile as tile
from concourse import bacc, mybir
from concourse.bass_utils import run_bass_kernel_spmd

B_PER_CORE = 2
D = 512
N = 2048
H = 8
NH = N // H  # 256
PT = D // 128  # 4 partition tiles over D
HT = NH // 128  # 2 partition tiles over one head's n-range

F32 = mybir.dt.float32
F32R = mybir.dt.float32r
F8 = mybir.dt.float8e4
WARM_F = 130  # warmup matmul free size (even, small init cost)
VPAD = 16  # v tile col padding: DoubleRow k-tile stride must be %16 bytes

_NC_CACHE = None


def build_nc():
    nc = bacc.Bacc("TRN2", target_bir_lowering=False, debug=False)

    # xq is loaded twice: f32r for the residual Z = XQ - attn, fp8 for the
    # QT projection (all of q/k/v/p~ run as fp8 DoubleRow matmuls; measured
    # end-to-end rel err ~7.6e-3 vs the 2e-2 gate).
    xq = nc.dram_tensor("xq", [B_PER_CORE, D, N], F32R, kind="ExternalInput").ap()
    xq8 = nc.dram_tensor("xq8", [B_PER_CORE, D, N], F8, kind="ExternalInput").ap()
    xk = nc.dram_tensor("xk", [B_PER_CORE, D, N], F8, kind="ExternalInput").ap()
    xv = nc.dram_tensor("xv", [B_PER_CORE, D, N], F8, kind="ExternalInput").ap()
    wqt = nc.dram_tensor("wqt", [D, D], F8, kind="ExternalInput").ap()
    wkt = nc.dram_tensor("wkt", [D, D], F8, kind="ExternalInput").ap()
    wvt = nc.dram_tensor("wvt", [D, D], F8, kind="ExternalInput").ap()
    wot = nc.dram_tensor("wot", [D, D], F32R, kind="ExternalInput").ap()
    # n-major output: out[b, n, e] = final[b, e, n]; host transposes back.
    out = nc.dram_tensor("out", [B_PER_CORE, N, D], F32, kind="ExternalOutput").ap()

    with tile.TileContext(nc) as tc:
        with (
            tc.tile_pool(name="wpool", bufs=1) as wpool,
            tc.tile_pool(name="zpool", bufs=2) as zpool,
            tc.tile_pool(name="xpool", bufs=3) as xpool,
            tc.tile_pool(name="qkpool", bufs=2) as qkpool,
            tc.tile_pool(name="vpool", bufs=2) as vpool,
            tc.tile_pool(name="ptpool", bufs=2) as ptpool,
            tc.tile_pool(name="opool", bufs=6) as opool,
            tc.tile_pool(name="rpool", bufs=6) as rpool,
            tc.tile_pool(name="psq", bufs=2, space="PSUM") as psq,
            tc.tile_pool(name="pss", bufs=2, space="PSUM") as pss,
            tc.tile_pool(name="pso", bufs=2, space="PSUM") as pso,
            tc.tile_pool(name="pst", bufs=1, space="PSUM") as pst,
        ):
            # Weights resident for the whole kernel: [p, it, o] = W.T[it*128+p, o]
            w_sb = {}
            w_dram = {"wq": wqt, "wk": wkt, "wv": wvt, "wo": wot}

            def load_w(name, parts=1):
                # parts>1 splits the load into multiple dma_starts: a single
                # dma_start's descriptors only spread over ~4-6 of the 16 DMA
                # queues, so chunking raises effective transfer bandwidth.
                w_sb[name] = wpool.tile(
                    [128, PT, D],
                    F32R if name == "wo" else F8,
                    name=f"w_{name}",
                    tag=f"w_{name}",
                )
                src = w_dram[name].rearrange("(t p) o -> p t o", p=128)
                step = PT // parts
                for c in range(parts):
                    sl = slice(c * step, (c + 1) * step)
                    nc.sync.dma_start(out=w_sb[name][:, sl, :], in_=src[:, sl, :])

            x_b = {
                "xq": [xq[b].rearrange("(t p) n -> p t n", p=128) for b in range(B_PER_CORE)],
                "xq8": [xq8[b].rearrange("(t p) n -> p t n", p=128) for b in range(B_PER_CORE)],
                "xk": [xk[b].rearrange("(t p) n -> p t n", p=128) for b in range(B_PER_CORE)],
                "xv": [xv[b].rearrange("(t p) n -> p t n", p=128) for b in range(B_PER_CORE)],
            }

            def load_x(b, h, nm, parts=1):
                ns_ = slice(h * NH, (h + 1) * NH)
                t = xpool.tile(
                    [128, PT, NH],
                    F32R if nm == "xq" else F8,
                    name=f"{nm}_h",
                    tag=f"{nm}_h",
                )
                step = PT // parts
                for c in range(parts):
                    sl = slice(c * step, (c + 1) * step)
                    nc.sync.dma_start(
                        out=t[:, sl, :], in_=x_b[nm][b][:, sl, ns_]
                    )
                return t

            def load_head(b, h):
                return [load_x(b, h, nm) for nm in ("xq", "xq8", "xk", "xv")]

            steps = [(b, h) for b in range(B_PER_CORE) for h in range(H)]
            head_tiles = {}

            warm = None

            def warmup(n):
                # Warmups allocate from pss (idle outside the scores phase)
                # so they never wait on the psq copy rotation.
                for _ in range(n):
                    ps_w = pss.tile([128, D], F32, name="ps_s", tag="ps_s")
                    nc.tensor.matmul(
                        ps_w[:, 0:WARM_F],
                        lhsT=warm[:, 0:128],
                        rhs=warm[:, 0:WARM_F],
                        start=True,
                        stop=True,
                    )

            def store_outT(b, h, jn, src_ps):
                """Copy one accumulated outT block to SBUF and DMA it out.

                The copy runs on Scalar: Vector is saturated with recip/STT
                during the O-loop, Scalar is idle there."""
                o_sb = opool.tile([128, D], F32, name="o_sb", tag="o_sb")
                nc.scalar.copy(out=o_sb, in_=src_ps)
                n0 = h * NH + jn * 128
                nc.sync.dma_start(out=out[b, n0 : n0 + 128, :], in_=o_sb)

            def emit_o_block(ob, oh, o_pt, o_v, o_xq):
                """O-matmul + Z + incremental n-major outT for head (ob, oh).

                Called one iteration AFTER the head's pt/v/xq were produced,
                so none of its PE work waits on Scalar/Vector."""
                z_h = zpool.tile([128, PT, NH], F32R, name="z_h", tag="z_h")
                psT = [
                    pst.tile([128, D], F32, name=f"ps_t{jn}", tag=f"ps_t{jn}")
                    for jn in range(HT)
                ]

                def outT_mm(dt_, stop):
                    for jn in range(HT):
                        nc.tensor.matmul(
                            psT[jn],
                            lhsT=z_h[:, dt_, jn * 128 : (jn + 1) * 128],
                            rhs=w_sb["wo"][:, dt_, :],
                            start=(dt_ == 0),
                            stop=stop,
                        )

                for dt_ in range(PT):
                    ps_o = pso.tile([128, NH + 2], F32, name="ps_o", tag="ps_o")
                    for etp in range(PT // 2):
                        nc.tensor.matmul(
                            ps_o,
                            lhsT=o_pt[
                                :, 2 * etp : 2 * etp + 2, dt_ * 128 : (dt_ + 1) * 128
                            ],
                            rhs=o_v[:, 2 * etp : 2 * etp + 2, 0 : NH + 2],
                            start=(etp == 0),
                            stop=(etp == PT // 2 - 1),
                            perf_mode=mybir.MatmulPerfMode.DoubleRow,
                        )
                    recip = rpool.tile([128, 1], F32, name="recip", tag="recip")
                    nc.vector.reciprocal(recip, ps_o[:, NH : NH + 1])
                    nc.vector.scalar_tensor_tensor(
                        out=z_h[:, dt_, :],
                        in0=ps_o[:, 0:NH],
                        scalar=recip,
                        in1=o_xq[:, dt_, :].bitcast(F32),
                        op0=mybir.AluOpType.mult,
                        op1=mybir.AluOpType.add,
                    )
                    # outT runs TWO dt behind: the Z-write STT on Vector lags
                    # the O PSUM group by ~600ns.
                    if dt_ >= 2:
                        outT_mm(dt_ - 2, stop=False)
                outT_mm(PT - 2, stop=False)
                outT_mm(PT - 1, stop=True)
                for jn in range(HT):
                    store_outT(ob, oh, jn, psT[jn])

            prev_ctx = None

            for idx, (b, h) in enumerate(steps):
                ns = slice(h * NH, (h + 1) * NH)

                if idx == 0:
                    # PE warmup: matmuls on dummy data during the initial DMA
                    # window hold the HAM clock ramp (otherwise the first
                    # ~3.4us run at 1.2GHz and the ramp restarts on idle).
                    # The init activation is kept tiny (130 cols) so the first
                    # warmup matmul starts ~300ns after Scalar boots.
                    warm = wpool.tile([128, 130], F32R, name="warm", tag="warm")
                    nc.scalar.activation(
                        out=warm,
                        in_=warm.bitcast(F32),
                        func=mybir.ActivationFunctionType.Copy,
                        bias=0.0,
                        scale=0.0,
                    )
                    # [128,1] constant -3.0 used as the exp bias AP.
                    # exp(s/16 - 3) keeps the fp8e4 outputs below 448 (the
                    # observed max at -3 is ~118; at -1 the data hits ~870
                    # and a single overflow NaNs a whole head via r).
                    neg_b = wpool.tile([128, 1], F32, name="negb", tag="negb")
                    nc.scalar.activation(
                        out=neg_b,
                        in_=warm[:, 0:1].bitcast(F32),
                        func=mybir.ActivationFunctionType.Copy,
                        bias=-3.0,
                        scale=0.0,
                    )
                    warmup(7)
                    # Priority-ordered chunked startup DMAs: the SP queue is
                    # in-order and the DMA queues are FIFO, so issue order is
                    # an effective bandwidth priority.  The fp8 weights are
                    # only 256KB, so QT's inputs (wq8+xq8 = 384KB) land
                    # ~1.1us after the first line.  xq (f32r, residual) is
                    # only needed by the O-block one iteration later.
                    load_w("wq", parts=2)
                    t_xq8 = load_x(0, 0, "xq8", parts=2)
                    load_w("wk", parts=2)
                    t_xk = load_x(0, 0, "xk", parts=2)
                    load_w("wv", parts=2)
                    t_xv = load_x(0, 0, "xv", parts=2)
                    # head 1's fp8 inputs next: iteration 1 starts ~7us in,
                    # while head 0's f32r xq (residual) and wo are only
                    # needed by the O-block one iteration later.
                    t1_xq8 = load_x(0, 1, "xq8")
                    t1_xk = load_x(0, 1, "xk")
                    t1_xv = load_x(0, 1, "xv")
                    t_xq = load_x(0, 0, "xq", parts=2)
                    load_w("wo", parts=2)
                    t1_xq = load_x(0, 1, "xq")
                    head_tiles[(0, 1)] = [t1_xq, t1_xq8, t1_xk, t1_xv]
                    head_tiles[(0, 0)] = [t_xq, t_xq8, t_xk, t_xv]

                xq_h, xq8_h, xk_h, xv_h = head_tiles.pop((b, h))
                # Prefetch the next head's inputs (heads 0 and 1 were
                # prefetched in the startup sequence).
                if idx >= 1 and idx + 1 < len(steps):
                    head_tiles[steps[idx + 1]] = load_head(*steps[idx + 1])

                # QT/KT: [p, jt, d] = X^T @ W^T (n-major projections), fp8
                # DoubleRow: each matmul contracts an it-pair (256 rows).
                qt_h = qkpool.tile([128, HT, D], F8, name="qt_h", tag="qt_h")
                kt_h = qkpool.tile([128, HT, D], F8, name="kt_h", tag="kt_h")
                for dst, xsrc, w in ((qt_h, xq8_h, "wq"), (kt_h, xk_h, "wk")):
                    for jt in range(HT):
                        ps = psq.tile([128, D], F32, name="ps_p", tag="ps_p")
                        for itp in range(PT // 2):
                            nc.tensor.matmul(
                                ps,
                                lhsT=xsrc[
                                    :, 2 * itp : 2 * itp + 2, jt * 128 : (jt + 1) * 128
                                ],
                                rhs=w_sb[w][:, 2 * itp : 2 * itp + 2, :],
                                start=(itp == 0),
                                stop=(itp == PT // 2 - 1),
                                perf_mode=mybir.MatmulPerfMode.DoubleRow,
                            )
                        # Vector, not Scalar: Scalar must stay free for exps.
                        nc.vector.tensor_copy(out=dst[:, jt, :], in_=ps)
                    if idx == 0 and dst is qt_h:
                        # wk/xk still in flight during head 0's QT.
                        warmup(2)

                # V (d-major): [p, et, n] in fp8e4; columns NH/NH+1 fixed at
                # -1.0 so the O-matmul accumulates -r in PSUM column NH.
                # fp8 DoubleRow contracts 2 k-tiles (256 rows) per matmul.
                v_h = vpool.tile([128, PT, NH + VPAD], F8, name="v_h", tag="v_h")
                # memset can't emit fp8; ACT Copy(in*0 - 1) = -1.0 can.
                nc.scalar.activation(
                    out=v_h[:, :, NH : NH + 2],
                    in_=warm[:, 0:8].rearrange("p (a b) -> p a b", a=PT).bitcast(F32),
                    func=mybir.ActivationFunctionType.Copy,
                    bias=-1.0,
                    scale=0.0,
                )

                def v_group(ep):
                    # An et-PAIR shares one [128,512] PSUM tile (each V
                    # output is only half a bank): halves the psq rotation
                    # pressure and the vector copy count.
                    ps = psq.tile([128, D], F32, name="ps_p", tag="ps_p")
                    for j in range(2):
                        et = 2 * ep + j
                        for itp in range(PT // 2):
                            nc.tensor.matmul(
                                ps[:, j * NH : j * NH + NH],
                                lhsT=w_sb["wv"][
                                    :, 2 * itp : 2 * itp + 2, et * 128 : (et + 1) * 128
                                ],
                                rhs=xv_h[:, 2 * itp : 2 * itp + 2, :],
                                start=(itp == 0),
                                stop=(itp == PT // 2 - 1),
                                perf_mode=mybir.MatmulPerfMode.DoubleRow,
                            )
                    nc.vector.tensor_copy(
                        out=v_h[:, 2 * ep : 2 * ep + 2, 0:NH],
                        in_=ps.rearrange("p (a n) -> p a n", a=2),
                    )

                # Head 0's V data lands after scores' inputs, so V runs after
                # scores there; otherwise V sits between KT and scores.
                if idx >= 1:
                    for ep in range(PT // 2):
                        v_group(ep)

                # scoresT (e-part, d-free) then p~ = exp(scoresT/16 - 3) fp8
                pt_t = ptpool.tile([128, PT, D], F8, name="pt_t", tag="pt_t")
                for et in range(PT):
                    ps_s = pss.tile([128, D], F32, name="ps_s", tag="ps_s")
                    nc.tensor.matmul(
                        ps_s,
                        lhsT=kt_h[:, 0:HT, et * 128 : (et + 1) * 128],
                        rhs=qt_h[:, 0:HT, :],
                        start=True,
                        stop=True,
                        perf_mode=mybir.MatmulPerfMode.DoubleRow,
                    )
                    nc.scalar.activation(
                        out=pt_t[:, et, :],
                        in_=ps_s,
                        func=mybir.ActivationFunctionType.Exp,
                        scale=float(1.0 / np.sqrt(NH)),
                        bias=neg_b,
                    )

                if idx == 0:
                    warmup(2)
                    for ep in range(PT // 2):
                        v_group(ep)

                # O + Z + outT for the PREVIOUS head: its pt/v/xq finished a
                # full iteration ago, so the PE never stalls on them.
                if prev_ctx is not None:
                    emit_o_block(*prev_ctx)
                prev_ctx = (b, h, pt_t, v_h, xq_h)

            emit_o_block(*prev_ctx)

    nc.compile()
    return nc


def _get_nc():
    global _NC_CACHE
    if _NC_CACHE is None:
        _NC_CACHE = build_nc()
    return _NC_CACHE


def _shard_inputs(inputs):
    f8 = ml_dtypes.float8_e4m3
    xq = np.ascontiguousarray(np.asarray(inputs["X_Query"], dtype=np.float32))
    xq8 = np.ascontiguousarray(xq.astype(f8))
    xk = np.ascontiguousarray(
        np.asarray(inputs["X_Key"], dtype=np.float32).astype(f8)
    )
    xv = np.ascontiguousarray(
        np.asarray(inputs["X_Value"], dtype=np.float32).astype(f8)
    )
    weights = {
        "wqt": np.ascontiguousarray(
            np.asarray(inputs["W_q"], dtype=np.float32).T.astype(f8)
        ),
        "wkt": np.ascontiguousarray(
            np.asarray(inputs["W_k"], dtype=np.float32).T.astype(f8)
        ),
        "wvt": np.ascontiguousarray(
            np.asarray(inputs["W_v"], dtype=np.float32).T.astype(f8)
        ),
        "wot": np.ascontiguousarray(np.asarray(inputs["W_o"], dtype=np.float32).T),
    }
    in_maps = []
    for c in range(8):
        sl = slice(c * B_PER_CORE, (c + 1) * B_PER_CORE)
        in_maps.append(
            {"xq": xq[sl], "xq8": xq8[sl], "xk": xk[sl], "xv": xv[sl], **weights}
        )
    return in_maps


def run_sharded(inputs, **kwargs):
    """Run on all 8 cores; returns (full_output, BassKernelResults)."""
    nc = _get_nc()
    in_maps = _shard_inputs(inputs)
    res = run_bass_kernel_spmd(nc, in_maps, core_ids=list(range(8)), **kwargs)
    # per-core out is [B_PER_CORE, N, D] (n-major); transpose back.
    full = np.concatenate(
        [np.ascontiguousarray(r["out"].transpose(0, 2, 1)) for r in res.results],
        axis=0,
    )
    return full, res


def kernel(**inputs):
    full, _ = run_sharded(inputs)
    return full
